# revision 12
# baseline (speedup 1.0000x reference)
"""Trainium2 Bass kernel for grouped-query attention with qk-norm.

Problem (hardcoded): x(2,2048,1024) @ Wq(1024,1024) / Wkv(1024,512),
16 query heads, 4 kv heads, head_dim 64, k_scale(16,1,64) applied to the
group-broadcast k. Output (2,2048,1024).

Sharding: 8 cores = batch(2) x kv_heads(4). Each core computes its batch's
4 query heads against its kv head over the full 2048x2048 score matrix.

Design (v3):
- k_scale is folded into Wq host-side (q_h.(ks_h*k) == (q_h*ks_h).k), so
  the device projects a SINGLE unscaled k (64 dims) shared by all 4 heads
  instead of 4 scaled copies: the k/v projections are packed into one
  [Wv|Wk] chain per chunk (halves projection matmul work).
- The packed chain yields psum rows 0:64 = vT, 64:128 = kT. kT lands in
  kkTrep rows 64:128 (half1 stationary); a DRAM bounce replicates it into
  rows 0:64 (DMA is the only cross-partition path).
- QK computes S^T (keys x queries) for two heads concurrently via
  tile_position row-packing (contraction is only d=64).
- exp work is split across TWO engines: ScalarE true exp (ACT is
  otherwise the serial bottleneck: 128 tiles x ~1.15us) and VectorE
  Schraudolph exp-approx in bf16: p = bitcast_bf16(int16(s*C + D)) - one
  tensor_scalar op per tile. The approx tiles raise the final rel err to
  ~9e-3 (measured vs reference; harness gate 2e-2).
- Softmax skips max-subtraction (scores bounded ~|7|); normalization
  happens after PV via an appended ones-row in the V stationary (psum row
  64 accumulates sum(p)).
- Schedule: attention for block (hp0,ic0)/(hp1,ic0) starts DURING the
  projection waves (held bf16 exp tiles; PV catches up once psum banks
  free), normalize/output-DMA is interleaved per block instead of at the
  end, and a longer dummy-matmul warmup keeps the PE HAM activity monitor
  from throttling the projection phase to 1.2 GHz.
"""

import os
from contextlib import ExitStack

import numpy as np

import concourse.bacc as bacc
import concourse.mybir as mybir
import concourse.tile as tile
from concourse.bass_utils import run_bass_kernel_spmd

# Problem constants
B, N, DIM = 2, 2048, 1024
HEADS, KV_HEADS, DH = 16, 4, 64
G = HEADS // KV_HEADS  # query heads per kv head (4)
NCORES = 8
P = 128
KT = DIM // P  # 8 contraction tiles over dim
IC = 512  # query-chunk width
NI = N // IC  # 4
NJ = N // P  # 16 key tiles
SCALE = DH**-0.5

F32 = mybir.dt.float32
F32R = mybir.dt.float32r
BF16 = mybir.dt.bfloat16
F16 = mybir.dt.float16
I16 = mybir.dt.int16

# Schraudolph bf16 exp-approx constants (offset tuned on host data)
LOG2E = 1.4426950408889634
SCH_C = float(SCALE * LOG2E * 128.0)
SCH_D = float(127 * 128 - 7.0)

# exp-engine assignment: which jt tiles of a full steady block go to the
# vector engine (Schraudolph). ~7/16 balances ACT vs DVE under a PE-bound
# schedule. Override with KERNEL_DVE=0 to force all-ACT (exact exp).
_DVE_ON = os.environ.get("KERNEL_DVE", "1") != "0"
DVE_JT = frozenset((1, 3, 5, 7, 9, 11, 13)) if _DVE_ON else frozenset()

WARMUP_MMS = 48


def emit_kernel(ctx, tc, xT, wq, wvk, eye, oT):
    nc = tc.nc
    Exp = mybir.ActivationFunctionType.Exp
    mult = mybir.AluOpType.mult
    add = mybir.AluOpType.add

    wpool = ctx.enter_context(tc.tile_pool(name="w", bufs=1))
    qkpool = ctx.enter_context(tc.tile_pool(name="qk", bufs=1))
    ptpool = ctx.enter_context(tc.tile_pool(name="pt", bufs=1))
    npool = ctx.enter_context(tc.tile_pool(name="norm", bufs=2))

    # --- persistent SBUF tensors ---
    ones_sb = wpool.tile([P, DH], F32R, tag="ones")  # warmup stationary
    eye_sb = wpool.tile([DH, DH], F32R, tag="eye")  # identity for vT transpose
    qT = [qkpool.tile([P, N], F32R, name=f"qT{hp}", tag=f"qT{hp}") for hp in range(2)]
    kkTrep = qkpool.tile([P, N], F32R, tag="kkTrep")  # k replicated in both halves
    vT_sb = qkpool.tile([DH, N], F32R, tag="vT")
    vaug = qkpool.tile([P, NJ * (DH + 1)], BF16, tag="vaug")
    nc.any.memset(vaug[:], 1.0)
    nc.any.memset(ones_sb[:].bitcast(F32), 1.0)
    # load the exp table set early (one-time ~2.7us)
    warm = qkpool.tile([1, 1], F32, tag="warm")
    nc.scalar.activation(warm[:], ones_sb[0:1, 0:1].bitcast(F32), Exp)
    nc.sync.dma_start(eye_sb[:], eye[:, :].bitcast(F32R))

    # DRAM scratch: k replication bounce + softmax-sum respread bounce
    kb_d = nc.dram_tensor("kb_d", (DH, N), F32, kind="ExternalOutput").ap()
    sums_d = nc.dram_tensor("sums_d", (G, N), F32, kind="ExternalOutput").ap()
    rec_d = nc.dram_tensor("rec_d", (G, N), F32, kind="ExternalOutput").ap()

    # --- attention helpers ---
    def qk_exp(hp, ic, jt, pt):
        """QK for both heads of pair hp (concurrent row-tiles) + exp -> pt."""
        csl = slice(ic * IC, (ic + 1) * IC)
        jsl = slice(jt * P, (jt + 1) * P)
        st = stpool.tile([P, 2 * IC], F32, tag="st", bufs=3, name="st")
        for half in range(2):
            rsl = slice(half * DH, half * DH + DH)
            nc.tensor.matmul(
                st[:, half * IC : (half + 1) * IC],
                kkTrep[rsl, jsl],
                qT[hp][rsl, csl],
                start=True,
                stop=True,
                tile_position=(half * DH, 0),
            )
        if pt.dtype == I16:
            # Schraudolph: bf16 bits = int16(s*C + D); includes the 1/8 scale
            nc.vector.tensor_scalar(pt[:], st[:], SCH_C, SCH_D, mult, add)
        else:
            nc.scalar.activation(pt[:], st[:], Exp, scale=SCALE)

    def pv_mm(o_ps, jt, pt, start, stop):
        ptb = pt.bitcast(BF16) if pt.dtype == I16 else pt
        for half in range(2):
            nc.tensor.matmul(
                o_ps[half][:],
                vaug[:, jt * (DH + 1) : (jt + 1) * (DH + 1)],
                ptb[:, half * IC : (half + 1) * IC],
                start=start,
                stop=stop,
            )

    def drain_norm(hp, ic, o_ps):
        """Copy PV out of psum, reciprocal of sums via DRAM respread,
        normalize and DMA the output chunk. Copies/multiplies run on
        gpsimd so they don't stall the vector engine's exp stream."""
        csl = slice(ic * IC, (ic + 1) * IC)
        for half in range(2):
            h = 2 * hp + half
            oacc = npool.tile([DH + 1, IC], F32, tag="oacc", bufs=4, name="oacc")
            nc.vector.tensor_copy(oacc[:], o_ps[half][:])
            nc.sync.dma_start(sums_d[h : h + 1, csl], oacc[DH : DH + 1, :])
            # respread [1,512] -> [128,4] (DVE recip on 1 partition is ~100x
            # slower), reciprocal, bounce back to a [1,512] row
            sums_t = npool.tile([P, 4], F32, tag="sums_t", bufs=2)
            rec_t = npool.tile([P, 4], F32, tag="rec_t", bufs=2)
            nc.sync.dma_start(
                sums_t[:], sums_d[h : h + 1, csl].rearrange("o (p f) -> (o p) f", p=P)
            )
            nc.vector.reciprocal(rec_t[:], sums_t[:])
            nc.sync.dma_start(
                rec_d[h : h + 1, csl].rearrange("o (p f) -> (o p) f", p=P), rec_t[:]
            )
            rec_row = npool.tile([1, IC], F32, tag="rec_row", bufs=4)
            nc.sync.dma_start(rec_row[:], rec_d[h : h + 1, csl])
            bc = npool.tile([DH, IC], F32, tag="bc", bufs=4)
            nc.gpsimd.partition_broadcast(bc[:], rec_row[0:1, :])
            fin = npool.tile([DH, IC], F32, tag="fin", bufs=4)
            nc.gpsimd.tensor_tensor(fin[:], oacc[0:DH, :], bc[:], mult)
            nc.sync.dma_start(oT[h * DH : (h + 1) * DH, csl], fin[:])

    # scores psum pool lives for the whole kernel (2 bufs x 2 banks)
    stpool = ctx.enter_context(tc.tile_pool(name="stp", bufs=3, space="PSUM"))

    # Dummy matmuls keep the PE HAM activity monitor busy through the
    # initial DMA wait so projections run at 2.4GHz instead of 1.2.
    for _ in range(WARMUP_MMS):
        wt = stpool.tile([DH, IC], F32, tag="st", name="wt", bufs=3)
        nc.tensor.matmul(
            wt[:, 0:DH], ones_sb[:, 0:DH], ones_sb[:, 0:DH], start=True, stop=True
        )

    # --- projections ---
    with tc.tile_pool(name="xw", bufs=1) as xwpool:
        wq_sb = xwpool.tile([P, KT * 256], F16, tag="wq")
        wvk_sb = xwpool.tile([P, KT * P], F16, tag="wvk")
        xts = xwpool.tile([P, KT * N], F16, tag="xt")  # 4MB

        def dma_x(kt, ic):
            r = slice(kt * P, (kt + 1) * P)
            csl = slice(ic * IC, (ic + 1) * IC)
            nc.gpsimd.dma_start(
                xts[:, kt * N + ic * IC : kt * N + (ic + 1) * IC], xT[r, csl]
            )

        for kt in range(KT):
            r = slice(kt * P, (kt + 1) * P)
            nc.sync.dma_start(wvk_sb[:, kt * P : (kt + 1) * P], wvk[r, :])
            dma_x(kt, 0)
        for kt in range(KT):
            r = slice(kt * P, (kt + 1) * P)
            nc.sync.dma_start(wq_sb[:, kt * 256 : (kt + 1) * 256], wq[r, :])
            dma_x(kt, 1)
        for kt in range(KT):
            dma_x(kt, 2)
        for kt in range(KT):
            dma_x(kt, 3)

        def proj_wave(ic, pp):
            csl = slice(ic * IC, (ic + 1) * IC)
            # packed [Wv|Wk] chain: psum rows 0:64 = vT, 64:128 = kT
            ps = pp.tile([P, IC], F32, tag="pj", name="pjkv", bufs=2)
            for kt in range(KT):
                nc.tensor.matmul(
                    ps[:],
                    wvk_sb[:, kt * P : (kt + 1) * P],
                    xts[:, kt * N + ic * IC : kt * N + (ic + 1) * IC],
                    start=(kt == 0),
                    stop=(kt == KT - 1),
                )
            nc.vector.tensor_copy(vT_sb[:, csl], ps[0:DH, :])
            nc.vector.tensor_copy(kkTrep[DH:P, csl], ps[DH:P, :])
            # replicate k into rows 0:64 through DRAM (cross-partition)
            nc.sync.dma_start(kb_d[:, csl].bitcast(F32R), kkTrep[DH:P, csl])
            nc.sync.dma_start(kkTrep[0:DH, csl], kb_d[:, csl].bitcast(F32R))
            # q chains (2 heads per chain)
            for hp in range(2):
                ps = pp.tile([P, IC], F32, tag="pj", name="pjq", bufs=2)
                for kt in range(KT):
                    c0 = kt * 256 + hp * P
                    nc.tensor.matmul(
                        ps[:],
                        wq_sb[:, c0 : c0 + P],
                        xts[:, kt * N + ic * IC : kt * N + (ic + 1) * IC],
                        start=(kt == 0),
                        stop=(kt == KT - 1),
                    )
                nc.vector.tensor_copy(qT[hp][:, csl], ps[:])
            # vaug (keys x d) via PE transpose
            for jt in range(4 * ic, 4 * ic + 4):
                pv = pp.tile([P, DH], F32R, tag="pj", bufs=2, name="pvt")
                nc.tensor.transpose(pv[:], vT_sb[:, jt * P : (jt + 1) * P], eye_sb[:])
                nc.vector.tensor_copy(
                    vaug[:, jt * (DH + 1) : jt * (DH + 1) + DH], pv[:].bitcast(F32)
                )

        # held exp tiles for blocks that start before psum banks free:
        # A=(hp0,ic0) jt0-11, B=(hp1,ic0) jt0-7 (all ACT/bf16)
        pt_hold = [
            ptpool.tile([P, 2 * IC], BF16, name=f"pth{j}", tag=f"pth{j}", bufs=1)
            for j in range(20)
        ]
        with tc.tile_pool(name="pp", bufs=2, space="PSUM") as pp:
            proj_wave(0, pp)
            for jt in range(4):  # A jt0-3
                qk_exp(0, 0, jt, pt_hold[jt])
            proj_wave(1, pp)
            for jt in range(4, 8):  # A jt4-7
                qk_exp(0, 0, jt, pt_hold[jt])
            for jt in range(4):  # B jt0-3
                qk_exp(1, 0, jt, pt_hold[12 + jt])
            proj_wave(2, pp)
            for jt in range(8, 12):  # A jt8-11
                qk_exp(0, 0, jt, pt_hold[jt])
            for jt in range(4, 8):  # B jt4-7
                qk_exp(1, 0, jt, pt_hold[12 + jt])
            proj_wave(3, pp)

    # --- steady-state attention: flat software pipeline ---
    # Jobs are (block, jt) pairs in block order. The QK+exp stream is
    # emitted LAG steady-jobs ahead of the PV stream so each PV's exp
    # result is ready when the PE reaches it (no head-of-line stall),
    # and block k+1's QKs overlap block k's PV/drain.
    blocks = [(0, 0), (1, 0), (0, 1), (1, 1), (0, 2), (1, 2), (0, 3), (1, 3)]
    jobs = []  # (bi, hp, ic, jt, held_pt_or_None)
    for bi, (hp, ic) in enumerate(blocks):
        for jt in range(NJ):
            held = None
            if bi == 0 and jt < 12:
                held = pt_hold[jt]
            elif bi == 1 and jt < 8:
                held = pt_hold[12 + jt]
            jobs.append((bi, hp, ic, jt, held))
    steady = [j for j in jobs if j[4] is None]
    LAG = 3
    pts = {}  # steady job -> pt tile
    s_cursor = 0

    def pump_qk(upto):
        nonlocal s_cursor
        while s_cursor < min(upto, len(steady)):
            bi, hp, ic, jt, _ = steady[s_cursor]
            dtype = I16 if (jt in DVE_JT) else BF16
            pt = ptpool.tile([P, 2 * IC], dtype, tag="ptr", bufs=6, name="ptr")
            qk_exp(hp, ic, jt, pt)
            pts[(bi, jt)] = pt
            s_cursor += 1

    with tc.tile_pool(name="op", bufs=1, space="PSUM") as opool:
        o_ps = None
        n_steady_done = 0
        for bi, hp, ic, jt, held in jobs:
            if jt == 0:
                o_ps = [
                    opool.tile(
                        [DH + 1, IC], F32, name=f"ops{i}", tag=f"ops{i}", bufs=1
                    )
                    for i in range(2)
                ]
            if held is None:
                n_steady_done += 1
            pump_qk(n_steady_done + LAG)
            pt = held if held is not None else pts.pop((bi, jt))
            pv_mm(o_ps, jt, pt, jt == 0, jt == NJ - 1)
            if jt == NJ - 1:
                drain_norm(hp, ic, o_ps)


_CACHE = {}


def build():
    if "nc" in _CACHE:
        return _CACHE["nc"]
    nc = bacc.Bacc(
        "TRN2", target_bir_lowering=False, debug=False, num_devices=NCORES
    )
    xT = nc.dram_tensor("xT", (DIM, N), F16, kind="ExternalInput").ap()
    wq = nc.dram_tensor("wq", (DIM, G * DH), F16, kind="ExternalInput").ap()
    wvk = nc.dram_tensor("wvk", (DIM, 2 * DH), F16, kind="ExternalInput").ap()
    eye = nc.dram_tensor("eye", (DH, DH), F32, kind="ExternalInput").ap()
    oT = nc.dram_tensor("oT", (G * DH, N), F32, kind="ExternalOutput").ap()
    with tile.TileContext(nc) as tc:
        with ExitStack() as ctx:
            emit_kernel(ctx, tc, xT, wq, wvk, eye, oT)
    nc.compile()
    _CACHE["nc"] = nc
    return nc


def make_in_maps(x, Wq, Wkv, k_scale):
    x = np.asarray(x, dtype=np.float32)
    Wq = np.asarray(Wq, dtype=np.float32)
    Wkv = np.asarray(Wkv, dtype=np.float32)
    k_scale = np.asarray(k_scale, dtype=np.float32)
    xTs = [np.ascontiguousarray(x[b].T) for b in range(B)]
    in_maps = []
    for c in range(NCORES):
        b, kv = divmod(c, KV_HEADS)
        # fold k_scale into Wq: q_h.(ks_h*k) == (q_h*ks_h).k
        wq_c = np.concatenate(
            [
                Wq[:, (kv * G + j) * DH : (kv * G + j + 1) * DH]
                * k_scale[kv * G + j, 0][None, :]
                for j in range(G)
            ],
            axis=1,
        )
        wvk_c = np.concatenate(
            [
                Wkv[:, KV_HEADS * DH + kv * DH : KV_HEADS * DH + (kv + 1) * DH],  # Wv
                Wkv[:, kv * DH : (kv + 1) * DH],  # Wk
            ],
            axis=1,
        )
        in_maps.append(
            {
                "xT": xTs[b].astype(np.float16),
                "wq": np.ascontiguousarray(wq_c).astype(np.float16),
                "wvk": np.ascontiguousarray(wvk_c).astype(np.float16),
                "eye": np.eye(DH, dtype=np.float32),
            }
        )
    return in_maps


def gather(results):
    out = np.empty((B, N, HEADS * DH), dtype=np.float32)
    for c in range(NCORES):
        b, kv = divmod(c, KV_HEADS)
        out[b, :, kv * G * DH : (kv + 1) * G * DH] = results[c]["oT"].T
    return out


def kernel(x, Wq, Wkv, k_scale, _trace=False):
    nc = build()
    in_maps = make_in_maps(x, Wq, Wkv, k_scale)
    res = run_bass_kernel_spmd(
        nc, in_maps, core_ids=list(range(NCORES)), trace=_trace
    )
    out = gather(res.results)
    if _trace:
        kernel.last_result = res
    return out


# revision 14
# speedup vs baseline: 1.7377x; 1.7377x over previous
"""Trainium2 Bass kernel for grouped-query attention with qk-norm.

Problem (hardcoded): x(2,2048,1024) @ Wq(1024,1024) / Wkv(1024,512),
16 query heads, 4 kv heads, head_dim 64, k_scale(16,1,64) applied to the
group-broadcast k. Output (2,2048,1024).

Sharding: 8 cores = batch(2) x kv_heads(4). Each core computes its batch's
4 query heads against its kv head over the full 2048x2048 score matrix.

Design (v3):
- k_scale is folded into Wq host-side (q_h.(ks_h*k) == (q_h*ks_h).k), so
  the device projects a SINGLE unscaled k (64 dims) shared by all 4 heads
  instead of 4 scaled copies: the k/v projections are packed into one
  [Wv|Wk] chain per chunk (halves projection matmul work).
- The packed chain yields psum rows 0:64 = vT, 64:128 = kT. kT lands in
  kkTrep rows 64:128 (half1 stationary); a DRAM bounce replicates it into
  rows 0:64 (DMA is the only cross-partition path).
- QK computes S^T (keys x queries) for two heads concurrently via
  tile_position row-packing (contraction is only d=64).
- exp work is split across TWO engines: ScalarE true exp (ACT is
  otherwise the serial bottleneck: 128 tiles x ~1.15us) and VectorE
  Schraudolph exp-approx in bf16: p = bitcast_bf16(int16(s*C + D)) - one
  tensor_scalar op per tile. The approx tiles raise the final rel err to
  ~9e-3 (measured vs reference; harness gate 2e-2).
- Softmax skips max-subtraction (scores bounded ~|7|); normalization
  happens after PV via an appended ones-row in the V stationary (psum row
  64 accumulates sum(p)).
- Schedule: attention for block (hp0,ic0)/(hp1,ic0) starts DURING the
  projection waves (held bf16 exp tiles; PV catches up once psum banks
  free), normalize/output-DMA is interleaved per block instead of at the
  end, and a longer dummy-matmul warmup keeps the PE HAM activity monitor
  from throttling the projection phase to 1.2 GHz.
"""

import os
from contextlib import ExitStack

import numpy as np

import concourse.bacc as bacc
import concourse.mybir as mybir
import concourse.tile as tile
from concourse.bass_utils import run_bass_kernel_spmd

# Problem constants
B, N, DIM = 2, 2048, 1024
HEADS, KV_HEADS, DH = 16, 4, 64
G = HEADS // KV_HEADS  # query heads per kv head (4)
NCORES = 8
P = 128
KT = DIM // P  # 8 contraction tiles over dim
IC = 512  # query-chunk width
NI = N // IC  # 4
NJ = N // P  # 16 key tiles
SCALE = DH**-0.5

F32 = mybir.dt.float32
F32R = mybir.dt.float32r
BF16 = mybir.dt.bfloat16
F16 = mybir.dt.float16
I16 = mybir.dt.int16

# Schraudolph bf16 exp-approx constants (offset tuned on host data)
LOG2E = 1.4426950408889634
SCH_C = float(SCALE * LOG2E * 128.0)
SCH_D = float(127 * 128 - 7.0)

# exp-engine assignment: which jt tiles of a full steady block go to the
# vector engine (Schraudolph). ~7/16 balances ACT vs DVE under a PE-bound
# schedule. Override with KERNEL_DVE=0 to force all-ACT (exact exp).
_DVE_ON = os.environ.get("KERNEL_DVE", "1") != "0"
DVE_JT = frozenset((1, 3, 5, 7, 9, 11, 13)) if _DVE_ON else frozenset()

WARMUP_MMS = 48


def emit_kernel(ctx, tc, xT, wq, wvk, eye, oT):
    nc = tc.nc
    Exp = mybir.ActivationFunctionType.Exp
    mult = mybir.AluOpType.mult
    add = mybir.AluOpType.add

    wpool = ctx.enter_context(tc.tile_pool(name="w", bufs=1))
    qkpool = ctx.enter_context(tc.tile_pool(name="qk", bufs=1))
    ptpool = ctx.enter_context(tc.tile_pool(name="pt", bufs=1))
    npool = ctx.enter_context(tc.tile_pool(name="norm", bufs=2))

    # --- persistent SBUF tensors ---
    ones_sb = wpool.tile([P, DH], F16, tag="ones")  # warmup stationary
    eye_sb = wpool.tile([DH, DH], F16, tag="eye")  # identity for vT transpose
    qT = [qkpool.tile([P, N], F16, name=f"qT{hp}", tag=f"qT{hp}") for hp in range(2)]
    kkTrep = qkpool.tile([P, N], F16, tag="kkTrep")  # k replicated in both halves
    vT_sb = qkpool.tile([DH, N], F16, tag="vT")
    vaug = qkpool.tile([P, NJ * (DH + 1)], BF16, tag="vaug")
    nc.any.memset(vaug[:], 1.0)
    nc.any.memset(ones_sb[:], 1.0)
    # load the exp table set early (one-time ~2.7us)
    warm = qkpool.tile([1, 1], F32, tag="warm")
    nc.scalar.activation(warm[:], ones_sb[0:1, 0:1], Exp)
    nc.sync.dma_start(eye_sb[:], eye[:, :])

    # DRAM scratch: k replication bounce + softmax-sum respread bounce
    kb_d = nc.dram_tensor("kb_d", (DH, N), F16, kind="ExternalOutput").ap()
    sums_d = nc.dram_tensor("sums_d", (G, N), F32, kind="ExternalOutput").ap()
    rec_d = nc.dram_tensor("rec_d", (G, N), F32, kind="ExternalOutput").ap()

    # --- attention helpers ---
    def qk_exp(hp, ic, jt, pt):
        """QK for both heads of pair hp (concurrent row-tiles) + exp -> pt."""
        csl = slice(ic * IC, (ic + 1) * IC)
        jsl = slice(jt * P, (jt + 1) * P)
        st = stpool.tile([P, 2 * IC], F32, tag="st", bufs=2, name="st")
        for half in range(2):
            rsl = slice(half * DH, half * DH + DH)
            nc.tensor.matmul(
                st[:, half * IC : (half + 1) * IC],
                kkTrep[rsl, jsl],
                qT[hp][rsl, csl],
                start=True,
                stop=True,
                tile_position=(half * DH, 0),
            )
        if pt.dtype == I16:
            # Schraudolph: bf16 bits = int16(s*C + D); includes the 1/8 scale
            nc.vector.tensor_scalar(pt[:], st[:], SCH_C, SCH_D, mult, add)
        else:
            nc.scalar.activation(pt[:], st[:], Exp, scale=SCALE)

    def pv_mm(o_ps, jt, pt, start, stop):
        ptb = pt.bitcast(BF16) if pt.dtype == I16 else pt
        for half in range(2):
            nc.tensor.matmul(
                o_ps[half][:],
                vaug[:, jt * (DH + 1) : (jt + 1) * (DH + 1)],
                ptb[:, half * IC : (half + 1) * IC],
                start=start,
                stop=stop,
            )

    def drain_norm(hp, ic, o_ps):
        """Copy PV out of psum, reciprocal of sums via DRAM respread,
        normalize and DMA the output chunk. Copies/multiplies run on
        gpsimd so they don't stall the vector engine's exp stream."""
        csl = slice(ic * IC, (ic + 1) * IC)
        for half in range(2):
            h = 2 * hp + half
            oacc = npool.tile([DH + 1, IC], F32, tag="oacc", bufs=4, name="oacc")
            nc.vector.tensor_copy(oacc[:], o_ps[half][:])
            nc.sync.dma_start(sums_d[h : h + 1, csl], oacc[DH : DH + 1, :])
            # respread [1,512] -> [128,4] (DVE recip on 1 partition is ~100x
            # slower), reciprocal, bounce back to a [1,512] row
            sums_t = npool.tile([P, 4], F32, tag="sums_t", bufs=2)
            rec_t = npool.tile([P, 4], F32, tag="rec_t", bufs=2)
            nc.sync.dma_start(
                sums_t[:], sums_d[h : h + 1, csl].rearrange("o (p f) -> (o p) f", p=P)
            )
            nc.vector.reciprocal(rec_t[:], sums_t[:])
            nc.sync.dma_start(
                rec_d[h : h + 1, csl].rearrange("o (p f) -> (o p) f", p=P), rec_t[:]
            )
            rec_row = npool.tile([1, IC], F32, tag="rec_row", bufs=4)
            nc.sync.dma_start(rec_row[:], rec_d[h : h + 1, csl])
            bc = npool.tile([DH, IC], F32, tag="bc", bufs=4)
            nc.gpsimd.partition_broadcast(bc[:], rec_row[0:1, :])
            fin = npool.tile([DH, IC], F32, tag="fin", bufs=4)
            nc.vector.tensor_tensor(fin[:], oacc[0:DH, :], bc[:], mult)
            nc.sync.dma_start(oT[h * DH : (h + 1) * DH, csl], fin[:])

    # scores psum pool lives for the whole kernel (2 bufs x 2 banks)
    stpool = ctx.enter_context(tc.tile_pool(name="stp", bufs=2, space="PSUM"))

    # Dummy matmuls keep the PE HAM activity monitor busy through the
    # initial DMA wait so projections run at 2.4GHz instead of 1.2.
    for _ in range(WARMUP_MMS):
        wt = stpool.tile([DH, IC], F32, tag="st", name="wt", bufs=2)
        nc.tensor.matmul(
            wt[:, 0:DH], ones_sb[:, 0:DH], ones_sb[:, 0:DH], start=True, stop=True
        )

    # --- projections ---
    with tc.tile_pool(name="xw", bufs=1) as xwpool:
        wq_sb = xwpool.tile([P, KT * 256], F16, tag="wq")
        wvk_sb = xwpool.tile([P, KT * P], F16, tag="wvk")
        xts = xwpool.tile([P, KT * N], F16, tag="xt")  # 4MB

        def dma_x(kt, ic):
            r = slice(kt * P, (kt + 1) * P)
            csl = slice(ic * IC, (ic + 1) * IC)
            nc.gpsimd.dma_start(
                xts[:, kt * N + ic * IC : kt * N + (ic + 1) * IC], xT[r, csl]
            )

        for kt in range(KT):
            r = slice(kt * P, (kt + 1) * P)
            nc.sync.dma_start(wvk_sb[:, kt * P : (kt + 1) * P], wvk[r, :])
            dma_x(kt, 0)
        for kt in range(KT):
            r = slice(kt * P, (kt + 1) * P)
            nc.sync.dma_start(wq_sb[:, kt * 256 : (kt + 1) * 256], wq[r, :])
            dma_x(kt, 1)
        for kt in range(KT):
            dma_x(kt, 2)
        for kt in range(KT):
            dma_x(kt, 3)

        def proj_wave(ic, pp):
            csl = slice(ic * IC, (ic + 1) * IC)
            # packed [Wv|Wk] chain: psum rows 0:64 = vT, 64:128 = kT
            ps = pp.tile([P, IC], F32, tag="pj", name="pjkv", bufs=2)
            for kt in range(KT):
                nc.tensor.matmul(
                    ps[:],
                    wvk_sb[:, kt * P : (kt + 1) * P],
                    xts[:, kt * N + ic * IC : kt * N + (ic + 1) * IC],
                    start=(kt == 0),
                    stop=(kt == KT - 1),
                )
            nc.vector.tensor_copy(vT_sb[:, csl], ps[0:DH, :])
            nc.vector.tensor_copy(kkTrep[DH:P, csl], ps[DH:P, :])
            # replicate k into rows 0:64 through DRAM (cross-partition)
            nc.sync.dma_start(kb_d[:, csl], kkTrep[DH:P, csl])
            nc.sync.dma_start(kkTrep[0:DH, csl], kb_d[:, csl])
            # q chains (2 heads per chain)
            for hp in range(2):
                ps = pp.tile([P, IC], F32, tag="pj", name="pjq", bufs=2)
                for kt in range(KT):
                    c0 = kt * 256 + hp * P
                    nc.tensor.matmul(
                        ps[:],
                        wq_sb[:, c0 : c0 + P],
                        xts[:, kt * N + ic * IC : kt * N + (ic + 1) * IC],
                        start=(kt == 0),
                        stop=(kt == KT - 1),
                    )
                nc.vector.tensor_copy(qT[hp][:, csl], ps[:])
            # vaug (keys x d) via PE transpose
            for jt in range(4 * ic, 4 * ic + 4):
                pv = pp.tile([P, DH], F16, tag="pj", bufs=2, name="pvt")
                nc.tensor.transpose(pv[:], vT_sb[:, jt * P : (jt + 1) * P], eye_sb[:])
                nc.vector.tensor_copy(
                    vaug[:, jt * (DH + 1) : jt * (DH + 1) + DH], pv[:]
                )

        # held exp tiles for blocks that start before psum banks free:
        # A=(hp0,ic0) jt0-11, B=(hp1,ic0) jt0-7 (all ACT/bf16)
        pt_hold = [
            ptpool.tile([P, 2 * IC], BF16, name=f"pth{j}", tag=f"pth{j}", bufs=1)
            for j in range(20)
        ]
        with tc.tile_pool(name="pp", bufs=2, space="PSUM") as pp:
            proj_wave(0, pp)
            for jt in range(4):  # A jt0-3
                qk_exp(0, 0, jt, pt_hold[jt])
            proj_wave(1, pp)
            for jt in range(4, 8):  # A jt4-7
                qk_exp(0, 0, jt, pt_hold[jt])
            for jt in range(4):  # B jt0-3
                qk_exp(1, 0, jt, pt_hold[12 + jt])
            proj_wave(2, pp)
            for jt in range(8, 12):  # A jt8-11
                qk_exp(0, 0, jt, pt_hold[jt])
            for jt in range(4, 8):  # B jt4-7
                qk_exp(1, 0, jt, pt_hold[12 + jt])
            proj_wave(3, pp)

    # --- steady-state attention: flat software pipeline ---
    # Jobs are (block, jt) pairs in block order. The QK+exp stream is
    # emitted LAG steady-jobs ahead of the PV stream so each PV's exp
    # result is ready when the PE reaches it (no head-of-line stall),
    # and block k+1's QKs overlap block k's PV/drain.
    blocks = [(0, 0), (1, 0), (0, 1), (1, 1), (0, 2), (1, 2), (0, 3), (1, 3)]
    jobs = []  # (bi, hp, ic, jt, held_pt_or_None)
    for bi, (hp, ic) in enumerate(blocks):
        for jt in range(NJ):
            held = None
            if bi == 0 and jt < 12:
                held = pt_hold[jt]
            elif bi == 1 and jt < 8:
                held = pt_hold[12 + jt]
            jobs.append((bi, hp, ic, jt, held))
    steady = [j for j in jobs if j[4] is None]
    LAG = 4
    pts = {}  # steady job -> pt tile
    s_cursor = 0

    def pump_qk(upto):
        nonlocal s_cursor
        while s_cursor < min(upto, len(steady)):
            bi, hp, ic, jt, _ = steady[s_cursor]
            dtype = I16 if (jt in DVE_JT) else BF16
            pt = ptpool.tile([P, 2 * IC], dtype, tag="ptr", bufs=6, name="ptr")
            qk_exp(hp, ic, jt, pt)
            pts[(bi, jt)] = pt
            s_cursor += 1

    with tc.tile_pool(name="op", bufs=2, space="PSUM") as opool:
        o_ps = None
        n_steady_done = 0
        for bi, hp, ic, jt, held in jobs:
            if jt == 0:
                o_ps = [
                    opool.tile(
                        [DH + 1, IC], F32, name=f"ops{i}", tag=f"ops{i}", bufs=2
                    )
                    for i in range(2)
                ]
            if held is None:
                n_steady_done += 1
            pump_qk(n_steady_done + LAG)
            pt = held if held is not None else pts.pop((bi, jt))
            pv_mm(o_ps, jt, pt, jt == 0, jt == NJ - 1)
            if jt == NJ - 1:
                drain_norm(hp, ic, o_ps)


_CACHE = {}


def build():
    if "nc" in _CACHE:
        return _CACHE["nc"]
    nc = bacc.Bacc(
        "TRN2", target_bir_lowering=False, debug=False, num_devices=NCORES
    )
    xT = nc.dram_tensor("xT", (DIM, N), F16, kind="ExternalInput").ap()
    wq = nc.dram_tensor("wq", (DIM, G * DH), F16, kind="ExternalInput").ap()
    wvk = nc.dram_tensor("wvk", (DIM, 2 * DH), F16, kind="ExternalInput").ap()
    eye = nc.dram_tensor("eye", (DH, DH), F16, kind="ExternalInput").ap()
    oT = nc.dram_tensor("oT", (G * DH, N), F32, kind="ExternalOutput").ap()
    with tile.TileContext(nc) as tc:
        with ExitStack() as ctx:
            emit_kernel(ctx, tc, xT, wq, wvk, eye, oT)
    nc.compile()
    _CACHE["nc"] = nc
    return nc


def make_in_maps(x, Wq, Wkv, k_scale):
    x = np.asarray(x, dtype=np.float32)
    Wq = np.asarray(Wq, dtype=np.float32)
    Wkv = np.asarray(Wkv, dtype=np.float32)
    k_scale = np.asarray(k_scale, dtype=np.float32)
    xTs = [np.ascontiguousarray(x[b].T) for b in range(B)]
    in_maps = []
    for c in range(NCORES):
        b, kv = divmod(c, KV_HEADS)
        # fold k_scale into Wq: q_h.(ks_h*k) == (q_h*ks_h).k
        wq_c = np.concatenate(
            [
                Wq[:, (kv * G + j) * DH : (kv * G + j + 1) * DH]
                * k_scale[kv * G + j, 0][None, :]
                for j in range(G)
            ],
            axis=1,
        )
        wvk_c = np.concatenate(
            [
                Wkv[:, KV_HEADS * DH + kv * DH : KV_HEADS * DH + (kv + 1) * DH],  # Wv
                Wkv[:, kv * DH : (kv + 1) * DH],  # Wk
            ],
            axis=1,
        )
        in_maps.append(
            {
                "xT": xTs[b].astype(np.float16),
                "wq": np.ascontiguousarray(wq_c).astype(np.float16),
                "wvk": np.ascontiguousarray(wvk_c).astype(np.float16),
                "eye": np.eye(DH, dtype=np.float16),
            }
        )
    return in_maps


def gather(results):
    out = np.empty((B, N, HEADS * DH), dtype=np.float32)
    for c in range(NCORES):
        b, kv = divmod(c, KV_HEADS)
        out[b, :, kv * G * DH : (kv + 1) * G * DH] = results[c]["oT"].T
    return out


def kernel(x, Wq, Wkv, k_scale, _trace=False):
    nc = build()
    in_maps = make_in_maps(x, Wq, Wkv, k_scale)
    res = run_bass_kernel_spmd(
        nc, in_maps, core_ids=list(range(NCORES)), trace=_trace
    )
    out = gather(res.results)
    if _trace:
        kernel.last_result = res
    return out
